# revision 33
# baseline (speedup 1.0000x reference)
"""Dynamic structural masking attention on 8 Trainium2 NeuronCores.

Reference computation (per batch b):
    sim  = cos_sim(x, x)                      [S, S]
    mask = sim > 0.7                          (shared across heads)
    q/k/v = x @ W.T + b, per-head split
    out  = softmax(where(mask, q k^T / 8, -inf)) @ v   [H, S, dk]

Sharding over 8 cores: batch (2) x head-group (2) x query-slice (2).
Each core computes, for its batch b, 8 heads and 1024 query rows:
  - Gram matrix G = x^T x rows for its queries (transposed layout), with
    norms folded into the threshold compare -> 0/1 mask tile (bf16).
    The query-block of the mask is symmetric; below-diagonal 512-spans
    are filled by bf16 xbar DMA-transposes instead of matmuls.
  - Projections QT/KT (transposed, bf16) and V (normal layout, fp32r)
    augmented with a ones column so the attention-weight row-sum
    (softmax denominator) falls out of the AV matmul for free.
  - Per head: scores^T = K Q^T per t-chunk, exp on ACT (scale=1/8),
    mask multiply on DVE, AV accumulate on PE (emission software-
    pipelined: AV lags scores; epilogues interleave into the next head;
    the K-projection is braided into this phase to fill PE idle time).
    The epilogue normalizes in transposed layout (reciprocal of the
    PSUM denominator row, GPSIMD partition-broadcast, one DVE multiply)
    and the host de-transposes the [dk, q] output slabs.

Matmuls run in fp32r (full PE rate at N>=256, ~1.5e-4 rel precision);
Q/K and scores use bf16 (their precision only shifts softmax weights
within the masked set). Key order per core is permuted so its query
slice occupies columns 0:SQ (attention is permutation-invariant over
keys) - the same SPMD program serves all cores with no dynamic offsets.
Cost-model timeline: ~312 us/core; verified vs the fp64 reference at
2.1e-4 max relative error on hardware.
"""

import numpy as np

# Problem dims (hardcoded per contract; kernel.py must be self-contained).
B = 2
S = 2048
D = 1024
H_TOT = 16
DK = 64
SIM_THRESH = 0.7
N_CORES = 8

_CACHE = {}


def _build(S_, D_, H_LOC, SQ, thresh, n_cores=N_CORES, debug_mask=False):
    """Build + compile the SPMD single-core program."""
    import concourse.bacc as bacc
    import concourse.mybir as mybir
    import concourse.tile as tile

    f32 = mybir.dt.float32
    f32r = mybir.dt.float32r
    bf16 = mybir.dt.bfloat16
    Alu = mybir.AluOpType
    Act = mybir.ActivationFunctionType

    JH = H_LOC * DK          # projection output cols per core
    ND = D_ // 128           # contraction chunks
    NT = S_ // 128           # key chunks
    NQ = SQ // 128           # query chunks
    NJ = JH // 128           # projection col chunks
    NSP = SQ // 512          # 512-wide spans over queries
    NKS = S_ // 512          # 512-wide spans over keys
    HPJ = 128 // DK          # heads per j-chunk
    assert SQ % 512 == 0 and S_ % 1024 == 0 and JH % 128 == 0

    nc = bacc.Bacc("TRN2", target_bir_lowering=False, debug=False,
                   num_devices=n_cores)

    xt_d = nc.dram_tensor("xt", [D_, S_], f32, kind="ExternalInput")
    wqt_d = nc.dram_tensor("wqt", [D_, JH], f32, kind="ExternalInput")
    wkt_d = nc.dram_tensor("wkt", [D_, JH], f32, kind="ExternalInput")
    wvt_d = nc.dram_tensor("wvt", [D_, JH], f32, kind="ExternalInput")
    bq_d = nc.dram_tensor("bq", [JH], f32, kind="ExternalInput")
    bk_d = nc.dram_tensor("bk", [JH], f32, kind="ExternalInput")
    bvb_d = nc.dram_tensor("bvb", [128, JH], f32, kind="ExternalInput")
    ones_d = nc.dram_tensor("ones1", [128, 1], f32, kind="ExternalInput")
    out_d = nc.dram_tensor("out", [H_LOC, DK, SQ], f32, kind="ExternalOutput")
    maskout_d = None
    if debug_mask:
        maskout_d = nc.dram_tensor("maskout", [S_, SQ], mybir.dt.bfloat16,
                                   kind="ExternalOutput")

    with tile.TileContext(nc) as tc:
        with (
            tc.tile_pool(name="small", bufs=1) as small,
            tc.tile_pool(name="mask", bufs=NT) as mask_pool,
            tc.tile_pool(name="qt", bufs=NJ) as qt_pool,
            tc.tile_pool(name="kt", bufs=NJ) as kt_pool,
            tc.tile_pool(name="vp", bufs=NT) as v_pool,
            tc.tile_pool(name="dram", bufs=1, space="DRAM") as dram,
        ):
            # --- persistent small tiles ---
            ones_t = small.tile([128, 1], f32r, tag="ones")
            nc.gpsimd.dma_start(ones_t[:], ones_d.ap())
            bq_t = small.tile([128, NJ], f32, tag="bq")
            nc.sync.dma_start(bq_t[:], bq_d.ap().rearrange("(c p) -> p c", p=128))
            bk_t = small.tile([128, NJ], f32, tag="bk")
            nc.sync.dma_start(bk_t[:], bk_d.ap().rearrange("(c p) -> p c", p=128))
            bvb_t = small.tile([128, JH], f32, tag="bvb")
            nc.sync.dma_start(bvb_t[:], bvb_d.ap())
            ones8_t = small.tile([128, H_LOC], f32, tag="ones8")
            nc.vector.memset(ones8_t[:], 1.0)
            dscr = dram.tile([1, S_], f32, tag="dscr")

            mask_t = [mask_pool.tile([128, SQ], bf16, tag="mask", name=f"mask{i}") for i in range(NT)]
            qt_t = [qt_pool.tile([128, SQ], bf16, tag="qt", name=f"qt{i}") for i in range(NJ)]
            kt_t = [kt_pool.tile([128, S_], bf16, tag="kt", name=f"kt{i}") for i in range(NJ)]
            v_t = [v_pool.tile([128, H_LOC, 65], f32r, tag="v", name=f"v{i}") for i in range(NT)]

            with (
                tc.tile_pool(name="xt", bufs=ND) as xt_pool,
                tc.tile_pool(name="thr", bufs=1) as thr_pool,
                tc.tile_pool(name="ps", bufs=3, space="PSUM") as ps,
            ):
                xt_t = [xt_pool.tile([128, S_], f32r, tag="xt", name=f"xtt{i}") for i in range(ND)]
                # query-slice columns first: G/QT/norm matmuls depend only on
                # cols 0:SQ plus each t-chunk's own columns, so PE starts as
                # soon as the first-half DMAs land
                for dc in range(ND):
                    nc.gpsimd.dma_start(xt_t[dc][:, 0:SQ],
                                        xt_d.ap()[dc * 128:(dc + 1) * 128, 0:SQ])
                if SQ < S_:
                    for dc in range(ND):
                        nc.gpsimd.dma_start(xt_t[dc][:, SQ:S_],
                                            xt_d.ap()[dc * 128:(dc + 1) * 128, SQ:S_])

                thrq_bc = thr_pool.tile([128, SQ], f32, tag="thrqbc")
                invnk_cols = thr_pool.tile([128, NT], f32, tag="invnkcols")

                # --- stage A: key norms via squares + ones-matmul reduce ---
                # processed in 1024-key groups so the first mask compares only
                # wait on first-half norms (second-half xt arrives later)
                with tc.tile_pool(name="sta", bufs=1) as sta:
                    nk_row = sta.tile([1, S_], f32, tag="nkrow")
                    thrq_row = sta.tile([1, SQ], f32, tag="thrqrow")
                    with tc.tile_pool(name="sqtmp", bufs=3) as sqp:
                        for grp in range(S_ // 1024):
                            for sp in (2 * grp, 2 * grp + 1):
                                n2_ps = ps.tile([128, 1024], f32, tag="ps")
                                for dc in range(ND):
                                    sq_t = sqp.tile([128, 512], f32r, tag="sq")
                                    nc.scalar.activation(
                                        sq_t[:],
                                        xt_t[dc][:, sp * 512:(sp + 1) * 512].bitcast(f32),
                                        Act.Square)
                                    nc.tensor.matmul(n2_ps[0:1, 0:512], ones_t[:],
                                                     sq_t[:], start=(dc == 0),
                                                     stop=(dc == ND - 1))
                                nc.scalar.activation(
                                    nk_row[0:1, sp * 512:(sp + 1) * 512],
                                    n2_ps[0:1, 0:512], Act.Sqrt)
                                if sp < NSP:
                                    nc.scalar.activation(
                                        thrq_row[0:1, sp * 512:(sp + 1) * 512],
                                        n2_ps[0:1, 0:512], Act.Sqrt,
                                        scale=thresh * thresh)
                            if grp == 0:
                                nc.gpsimd.partition_broadcast(thrq_bc[:], thrq_row[:])
                            a, b = grp * 1024, (grp + 1) * 1024
                            nc.vector.reciprocal(nk_row[0:1, a:b], nk_row[0:1, a:b])
                            nc.sync.dma_start(dscr[0:1, a:b], nk_row[0:1, a:b])
                            nc.sync.dma_start(
                                invnk_cols[:, grp * 8:(grp + 1) * 8],
                                dscr[0:1, a:b].rearrange("o (c p) -> (o p) c", p=128))

                # --- stage B: Gram rows -> mask; Q projection ---
                # The [keys 0:SQ, queries 0:SQ] block of the mask is
                # symmetric (queries are keys 0:SQ in core-local order), so
                # below-diagonal 256-spans are filled by bf16 xbar
                # DMA-transposes of already-computed tiles instead of
                # Gram matmuls.
                NQT = SQ // 128  # tiles whose keys lie in the query slice
                for tcn in range(NT):
                    sav = tcn // 4 if tcn < NQT else 0  # saved 512-spans
                    col0 = sav * 512
                    g_ps = ps.tile([128, 1024], f32, tag="ps")
                    for dc in range(ND):
                        for sp in range((SQ - col0) // 512):
                            a = col0 + sp * 512
                            nc.tensor.matmul(
                                g_ps[:, a:a + 512],
                                xt_t[dc][:, tcn * 128:(tcn + 1) * 128],
                                xt_t[dc][:, a:a + 512],
                                start=(dc == 0), stop=(dc == ND - 1))
                    # mask[k, q] = (G * (1/|x_k|)) > 0.7*|x_q|
                    nc.vector.scalar_tensor_tensor(
                        mask_t[tcn][:, col0:SQ], g_ps[:, col0:SQ],
                        invnk_cols[:, tcn:tcn + 1],
                        thrq_bc[:, col0:SQ], op0=Alu.mult, op1=Alu.is_gt)
                    for m in range(4 * sav):
                        nc.sync.dma_start(
                            mask_t[tcn][:, m * 128:(m + 1) * 128],
                            mask_t[m][:, tcn * 128:(tcn + 1) * 128],
                            transpose=True)
                    if maskout_d is not None:
                        nc.sync.dma_start(
                            maskout_d.ap()[tcn * 128:(tcn + 1) * 128, :],
                            mask_t[tcn][:])

                with tc.tile_pool(name="wq", bufs=ND) as wqp:
                    wq_c = []
                    for dc in range(ND):
                        wt = wqp.tile([128, JH], f32r, tag="w", name=f"wq{dc}")
                        nc.gpsimd.dma_start(wt[:],
                                            wqt_d.ap()[dc * 128:(dc + 1) * 128, :])
                        wq_c.append(wt)
                    for jc in range(NJ):
                        q_ps = ps.tile([128, 1024], f32, tag="ps")
                        for dc in range(ND):
                            for sp in range(NSP):
                                nc.tensor.matmul(
                                    q_ps[:, sp * 512:(sp + 1) * 512],
                                    wq_c[dc][:, jc * 128:(jc + 1) * 128],
                                    xt_t[dc][:, sp * 512:(sp + 1) * 512],
                                    start=(dc == 0), stop=(dc == ND - 1))
                        nc.scalar.activation(qt_t[jc][:], q_ps[:, 0:SQ], Act.Identity,
                                             bias=bq_t[:, jc:jc + 1])

                # --- stage C: K^T and V projections ---
                with tc.tile_pool(name="wv", bufs=ND) as wvp:
                    wv_c = []
                    for dc in range(ND):
                        wt = wvp.tile([128, JH], f32r, tag="w", name=f"wv{dc}")
                        nc.gpsimd.dma_start(wt[:],
                                            wvt_d.ap()[dc * 128:(dc + 1) * 128, :])
                        wv_c.append(wt)
                    for sc in range(NT):
                        v_ps = ps.tile([128, 1024], f32, tag="ps")
                        for dc in range(ND):
                            nc.tensor.matmul(
                                v_ps[:, 0:JH],
                                xt_t[dc][:, sc * 128:(sc + 1) * 128],
                                wv_c[dc][:],
                                start=(dc == 0), stop=(dc == ND - 1))
                        nc.vector.tensor_tensor(
                            v_t[sc][:, :, 0:64],
                            v_ps[:, 0:JH].rearrange("p (h e) -> p h e", h=H_LOC),
                            bvb_t[:].rearrange("p (h e) -> p h e", h=H_LOC),
                            op=Alu.add)
                        nc.vector.tensor_copy(v_t[sc][:, :, 64], ones8_t[:])

            # --- stage D: per-head masked attention ---
            with (
                tc.tile_pool(name="p", bufs=5) as p_pool,
                tc.tile_pool(name="osb", bufs=1) as out_pool,
                tc.tile_pool(name="rec", bufs=1) as rec_pool,
                tc.tile_pool(name="bc", bufs=1) as bc_pool,
                tc.tile_pool(name="wk", bufs=ND) as wkp,
                tc.tile_pool(name="scps", bufs=3, space="PSUM") as scps,
                tc.tile_pool(name="avps", bufs=1, space="PSUM") as avps,
            ):
                wk_c = []
                for dc in range(ND):
                    wt = wkp.tile([128, JH], f32r, tag="w", name=f"wk{dc}")
                    nc.gpsimd.dma_start(wt[:],
                                        wkt_d.ap()[dc * 128:(dc + 1) * 128, :])
                    wk_c.append(wt)

                def emit_kt(jc):
                    for half in range(S_ // 1024):
                        k_ps = scps.tile([128, 1024], f32, tag="sc",
                                         name=f"kps{jc}_{half}")
                        for dc in range(ND):
                            for sp in range(2):
                                o = half * 1024 + sp * 512
                                nc.tensor.matmul(
                                    k_ps[:, sp * 512:(sp + 1) * 512],
                                    wk_c[dc][:, jc * 128:(jc + 1) * 128],
                                    xt_t[dc][:, o:o + 512],
                                    start=(dc == 0), stop=(dc == ND - 1))
                        nc.scalar.activation(
                            kt_t[jc][:, half * 1024:(half + 1) * 1024],
                            k_ps[:], Act.Identity, bias=bk_t[:, jc:jc + 1])
                av_ps_of = {}

                LAG = 4  # av emission lags scores so PE never head-blocks

                def emit_scores(h, tcn):
                    jc = h // HPJ
                    ho = (h % HPJ) * DK
                    s_ps = scps.tile([128, 1024], f32, tag="sc",
                                     name=f"sps{h}_{tcn}")
                    for sp in range(NSP):
                        nc.tensor.matmul(
                            s_ps[:, sp * 512:(sp + 1) * 512],
                            kt_t[jc][ho:ho + DK, tcn * 128:(tcn + 1) * 128],
                            qt_t[jc][ho:ho + DK, sp * 512:(sp + 1) * 512],
                            start=True, stop=True)
                    p_t = p_pool.tile([128, SQ], f32r, tag="p",
                                      name=f"p{h}_{tcn}")
                    nc.scalar.activation(p_t[:], s_ps[:, 0:SQ], Act.Exp,
                                         scale=0.125)
                    nc.vector.tensor_tensor(p_t[:], p_t[:].bitcast(f32),
                                            mask_t[tcn][:], op=Alu.mult)
                    return p_t

                def emit_av(h, tcn, p_t):
                    av_ps = av_ps_of[h]
                    for sp in range(NSP):
                        nc.tensor.matmul(
                            av_ps[:, sp * 512:(sp + 1) * 512],
                            v_t[tcn][:, h, :],
                            p_t[:, sp * 512:(sp + 1) * 512],
                            start=(tcn == 0), stop=(tcn == NT - 1))

                def head_chunks(h, tcns):
                    for tcn in tcns:
                        p_t = emit_scores(h, tcn)
                        pending.append((h, tcn, p_t))
                        while len(pending) > LAG:
                            nc_h, nc_t, nc_p = pending.pop(0)
                            emit_av(nc_h, nc_t, nc_p)

                def head_epilogue(h):
                    av_ps = av_ps_of.pop(h)
                    rec_row = rec_pool.tile([1, SQ], f32, tag="rec",
                                            name=f"recrow{h}")
                    nc.vector.reciprocal(rec_row[:], av_ps[64:65, :])
                    rec_bc = bc_pool.tile([DK, SQ], f32, tag="bc",
                                          name=f"recbc{h}")
                    nc.gpsimd.partition_broadcast(rec_bc[:], rec_row[:])
                    o_t = out_pool.tile([DK, SQ], f32, tag="o", name=f"o{h}")
                    nc.vector.tensor_tensor(o_t[:], av_ps[0:DK, :], rec_bc[:],
                                            op=Alu.mult)
                    nc.sync.dma_start(out_d.ap()[h], o_t[:])

                # software-pipelined: head h-1's epilogue lands after head h's
                # first chunks so the PSUM->SBUF copy never stalls ACT
                pending = []
                PRO = max(LAG + 1, NT // 4)
                for h in range(H_LOC):
                    if h % HPJ == 0:
                        emit_kt(h // HPJ)
                    av_ps_of[h] = avps.tile([65, SQ], f32, tag="av",
                                            name=f"avps{h}")
                    head_chunks(h, range(0, PRO))
                    if h > 0:
                        head_epilogue(h - 1)
                    head_chunks(h, range(PRO, NT))
                while pending:
                    nc_h, nc_t, nc_p = pending.pop(0)
                    emit_av(nc_h, nc_t, nc_p)
                head_epilogue(H_LOC - 1)

    nc.compile()
    return nc


def _get_nc():
    key = (S, D, H_TOT, SIM_THRESH)
    if key not in _CACHE:
        _CACHE[key] = _build(S, D, 8, 1024, SIM_THRESH)
    return _CACHE[key]


def make_in_maps(x, Wq, bq, Wk, bk, Wv, bv, h_loc=8, sq=1024, n_cores=N_CORES):
    """Per-core input dicts. Core c: batch, head-group, query-slice; its
    keys are rolled so the query slice comes first."""
    x = np.asarray(x, dtype=np.float32)
    Wq, Wk, Wv = (np.asarray(w, dtype=np.float32) for w in (Wq, Wk, Wv))
    bq, bk, bv = (np.asarray(v_, dtype=np.float32) for v_ in (bq, bk, bv))
    jh = h_loc * DK
    seq = x.shape[1]
    d_model = x.shape[2]
    ones1 = np.ones((128, 1), np.float32)
    n_hg = d_model // jh
    n_qs = seq // sq
    in_maps = []
    for c in range(n_cores):
        b = c // (n_hg * n_qs)
        hg = (c % (n_hg * n_qs)) // n_qs
        qs = c % n_qs
        xb = x[b]
        order = np.concatenate([
            np.arange(qs * sq, (qs + 1) * sq),
            np.delete(np.arange(seq), np.s_[qs * sq:(qs + 1) * sq])])
        in_maps.append({
            "xt": np.ascontiguousarray(xb[order].T),
            "wqt": np.ascontiguousarray(Wq[hg * jh:(hg + 1) * jh].T),
            "wkt": np.ascontiguousarray(Wk[hg * jh:(hg + 1) * jh].T),
            "wvt": np.ascontiguousarray(Wv[hg * jh:(hg + 1) * jh].T),
            "bq": np.ascontiguousarray(bq[hg * jh:(hg + 1) * jh]),
            "bk": np.ascontiguousarray(bk[hg * jh:(hg + 1) * jh]),
            "bvb": np.ascontiguousarray(
                np.broadcast_to(bv[hg * jh:(hg + 1) * jh], (128, jh))),
            "ones1": ones1,
        })
    return in_maps


def assemble(results, h_tot=H_TOT, seq=S, h_loc=8, sq=1024, n_cores=N_CORES):
    n_hg = h_tot // h_loc
    n_qs = seq // sq
    n_b = n_cores // (n_hg * n_qs)
    out = np.empty((n_b, h_tot, seq, DK), np.float32)
    for c in range(n_cores):
        b = c // (n_hg * n_qs)
        hg = (c % (n_hg * n_qs)) // n_qs
        qs = c % n_qs
        out[b, hg * h_loc:(hg + 1) * h_loc, qs * sq:(qs + 1) * sq, :] = \
            results[c]["out"].transpose(0, 2, 1)
    return out


def kernel(x, Wq, bq, Wk, bk, Wv, bv, _trace=False):
    from concourse.bass_utils import run_bass_kernel_spmd
    nc = _get_nc()
    in_maps = make_in_maps(x, Wq, bq, Wk, bk, Wv, bv)
    res = run_bass_kernel_spmd(nc, in_maps, core_ids=list(range(N_CORES)),
                               trace=_trace)
    out = assemble(res.results)
    if _trace:
        return out, res
    return out
